# revision 2
# baseline (speedup 1.0000x reference)
"""Trainium2 Bass kernel: 3x3 same conv (B=16, C=256, H=W=112) + bias via
1D Winograd F(2,3) along x, bf16, data-parallel over batch (2 images/core).

Per output-row pair column t: 4 transform points p with
  v0=d0-d2, v1=d1+d2, v2=d2-d1, v3=d1-d3   (d_j = x[2t-1+j], zero-padded)
  M[p][y] = sum_{ky,cin} U[p,ky,cin,:] * V_p[y-1+ky]   (PSUM f32 accum)
  out[2t]   = M0+M1+M2+bias,  out[2t+1] = M1-M2-M3+bias
U = G @ w_row is precomputed on host (f64->bf16), G=[[1,0,0],[.5,.5,.5],
[.5,-.5,.5],[0,0,1]]. PE work drops 9/6 = 1.5x vs direct conv; V production
and the 4-op output transform run on DVE (scalar_tensor_tensor, bias fused).

Host also pre-deinterleaves input columns (even/odd phases, zero-padded to
57+57 per row, bf16) so every DVE read is packed stride-1 bf16 (2 elem/cyc).

Layout per core: 4 strips of 28 output rows; V strip buffers [128,4,30,56]
bf16 per (strip, in-half); 4 PSUM planes of [128,7,56] f32 per group of 7
output rows (8-bank pool = 2 groups in flight). V for strip s+1 is issued
on DVE before the transforms of strip s so the PE never waits at strip
boundaries. Input DMAs ride the scalar-engine ring, outputs the sync ring.
Output is written bf16 and upcast on host.
"""
import numpy as np
import ml_dtypes

from concourse import bacc, bass, mybir, tile
from concourse.bass_utils import run_bass_kernel_spmd

B, C, H, Wd = 16, 256, 112, 112
NCORES = 8
BPC = B // NCORES
RS = 28                  # output rows per strip
NSTRIP = H // RS         # 4
VROWS = RS + 2           # V rows per strip (halo)
NT = 56                  # tiles (output column pairs) per row
NG = 4                   # groups of 7 output rows per strip
GR = RS // NG            # 7
NP = GR * NT             # 392 positions per psum plane
PW = 114                 # packed input row: 57 even + 57 odd phase cols
f32 = mybir.dt.float32
bf16 = mybir.dt.bfloat16
ADD = mybir.AluOpType.add
SUB = mybir.AluOpType.subtract
MULT = mybir.AluOpType.mult


def build(repeat: int = 1, no_in: bool = False, no_out: bool = False,
          no_mm: bool = False, no_v: bool = False, no_tr: bool = False):
    nc = bacc.Bacc("TRN2", debug=False)
    inp_d = nc.dram_tensor("inp", [BPC, C, H, PW], bf16, kind="ExternalInput").ap()
    w_d = nc.dram_tensor("w", [4, 3, C, C], bf16, kind="ExternalInput").ap()
    bias_d = nc.dram_tensor("bias", [C, 1], f32, kind="ExternalInput").ap()
    out_d = nc.dram_tensor("out", [BPC, C, H, Wd], bf16, kind="ExternalOutput").ap()

    with tile.TileContext(nc) as tc:
        with (
            tc.tile_pool(name="wpool", bufs=1) as wp,
            tc.tile_pool(name="vpool", bufs=1) as vp,
            tc.tile_pool(name="ppool", bufs=4) as ptp,
            tc.tile_pool(name="opool", bufs=4) as op,
            tc.tile_pool(name="tpool", bufs=16) as tp,
            tc.tile_pool(name="pspool", bufs=8, space=bass.MemorySpace.PSUM) as pp,
        ):
            # U weight tiles: lhsT [K=cin-half, M=cout-half] per (p, ky, kh, mh)
            wt = {}
            for p in range(4):
                for ky in range(3):
                    for kh in range(2):
                        for mh in range(2):
                            t = wp.tile([128, 128], bf16, name=f"w{p}{ky}{kh}{mh}")
                            nc.sync.dma_start(
                                t[:], w_d[p, ky, kh * 128:(kh + 1) * 128,
                                          mh * 128:(mh + 1) * 128])
                            wt[p, ky, kh, mh] = t
            biases = []
            for mh in range(2):
                bt = wp.tile([128, 1], f32, name=f"bias{mh}")
                nc.sync.dma_start(bt[:], bias_d[mh * 128:(mh + 1) * 128, :])
                biases.append(bt)

            # V strip buffers (persistent per (s, kh); fully rewritten per use)
            vbufs = {}
            for s in range(NSTRIP):
                for kh in range(2):
                    vbufs[s, kh] = vp.tile([128, 4, VROWS, NT], bf16,
                                           name=f"v{s}{kh}")

            def load_and_v(img, s):
                """DMA packed input rows for strip s and produce its V planes."""
                ys = s * RS
                y0 = max(ys - 1, 0)
                y1 = min(ys + RS + 1, H)
                r0 = y0 - (ys - 1)
                nrows = y1 - y0
                for kh in range(2):
                    pt = ptp.tile([128, VROWS, PW], bf16, name="pt", tag="pt",
                                  bufs=4)
                    if not no_in:
                        nc.scalar.dma_start(
                            pt[:, r0:r0 + nrows, :],
                            inp_d[img, kh * 128:(kh + 1) * 128, y0:y1, :])
                    if s == 0:
                        nc.vector.memset(pt[:, 0:1, :], 0.0)
                    if s == NSTRIP - 1:
                        nc.vector.memset(pt[:, VROWS - 1:VROWS, :], 0.0)
                    if no_v:
                        continue
                    pe0 = pt[:, :, 0:NT]          # pe[t],  t=0..55
                    pe1 = pt[:, :, 1:NT + 1]      # pe[t+1]
                    po0 = pt[:, :, 57:57 + NT]    # po[t]
                    po1 = pt[:, :, 58:58 + NT]    # po[t+1]
                    # plain tensor_tensor (not scalar_tensor_tensor): only the
                    # former is eligible for the DVE 2x packed-bf16 mode
                    v = vbufs[s, kh]
                    nc.vector.tensor_sub(v[:, 0], pe0, pe1)
                    nc.vector.tensor_add(v[:, 1], po0, pe1)
                    nc.vector.tensor_sub(v[:, 2], pe1, po0)
                    nc.vector.tensor_sub(v[:, 3], po0, po1)

            def body():
                seq = [(img, s) for img in range(BPC) for s in range(NSTRIP)]
                load_and_v(*seq[0])
                for idx, (img, s) in enumerate(seq):
                    if idx + 1 < len(seq):
                        load_and_v(*seq[idx + 1])
                    ys = s * RS
                    for mh in range(2):
                        ot = op.tile([128, RS, Wd], bf16, name="ot", tag="ot")
                        oi = ot[:].rearrange("p r (t two) -> p r t two", two=2)
                        for g in range(NG):
                            ps = [pp.tile([128, GR, NT], f32, name="ps",
                                          tag="ps") for _ in range(4)]
                            if not no_mm:
                                for p in range(4):
                                    for c in range(6):
                                        ky, kh = c // 2, c % 2
                                        nc.tensor.matmul(
                                            ps[p][:], wt[p, ky, kh, mh][:],
                                            vbufs[s, kh][:, p,
                                                         GR * g + ky:
                                                         GR * g + ky + GR, :],
                                            start=(c == 0), stop=(c == 5))
                            if no_tr:
                                continue
                            # Act drains all four PSUM planes to bf16 SBUF
                            # (bias fused into M1): frees PSUM early and every
                            # DVE op below is all-SBUF packed bf16 (2x mode).
                            # (walrus also forbids >1 PSUM operand per op.)
                            a = tp.tile([128, GR, NT], bf16, name="a",
                                        tag="tmp")
                            nc.scalar.activation(
                                a[:], ps[1][:],
                                mybir.ActivationFunctionType.Identity,
                                bias=biases[mh][:])
                            mb = []
                            for p in (0, 2, 3):
                                m = tp.tile([128, GR, NT], bf16, name=f"m{p}",
                                            tag="tmp")
                                nc.scalar.activation(
                                    m[:], ps[p][:],
                                    mybir.ActivationFunctionType.Copy)
                                mb.append(m)
                            m0b, m2b, m3b = mb
                            te = tp.tile([128, GR, NT], bf16, name="te",
                                         tag="tmp")
                            nc.vector.tensor_add(te[:], a[:], m2b[:])
                            nc.vector.tensor_add(
                                oi[:, GR * g:GR * g + GR, :, 0], te[:],
                                m0b[:])
                            to = tp.tile([128, GR, NT], bf16, name="to",
                                         tag="tmp")
                            nc.vector.tensor_sub(to[:], a[:], m2b[:])
                            nc.vector.tensor_sub(
                                oi[:, GR * g:GR * g + GR, :, 1], to[:],
                                m3b[:])
                        if not no_out:
                            nc.sync.dma_start(
                                out_d[img, mh * 128:(mh + 1) * 128,
                                      ys:ys + RS, :]
                                .rearrange("p r c -> p (r c)"),
                                ot[:].rearrange("p r c -> p (r c)"))

            if repeat > 1:
                with tc.For_i(0, repeat, 1):
                    body()
            else:
                body()

    nc.compile()
    return nc


_NC = None
_G = np.array([[1, 0, 0], [0.5, 0.5, 0.5], [0.5, -0.5, 0.5], [0, 0, 1]],
              dtype=np.float64)


def _prep(inp, W, bias):
    U = np.einsum("pk,oiyk->pyio", _G, np.asarray(W, dtype=np.float64))
    U = np.ascontiguousarray(U.astype(ml_dtypes.bfloat16))
    x = np.asarray(inp, dtype=np.float32).astype(ml_dtypes.bfloat16)
    p = np.zeros((B, C, H, Wd + 3), dtype=ml_dtypes.bfloat16)
    p[..., 1:113] = x
    pk = np.concatenate([p[..., 0::2][..., :57], p[..., 1::2][..., :57]],
                        axis=-1)
    pk = np.ascontiguousarray(pk)
    bias_r = np.ascontiguousarray(
        np.asarray(bias, dtype=np.float32).reshape(C, 1))
    return pk, U, bias_r


def kernel(inp, W, bias):
    global _NC
    if _NC is None:
        _NC = build()
    pk, U, bias_r = _prep(inp, W, bias)
    in_maps = [
        {"inp": pk[c * BPC:(c + 1) * BPC], "w": U, "bias": bias_r}
        for c in range(NCORES)
    ]
    res = run_bass_kernel_spmd(_NC, in_maps, list(range(NCORES)))
    out = np.concatenate(
        [np.asarray(res.results[c]["out"]) for c in range(NCORES)], axis=0)
    return out.astype(np.float32)


# revision 3
# speedup vs baseline: 1.1793x; 1.1793x over previous
"""Trainium2 Bass kernel: 3x3 same conv (B=16, C=256, H=W=112) + bias via
1D Winograd F(2,3) along x, bf16, data-parallel over batch (2 images/core).

Per output-row pair column t: 4 transform points p with
  v0=d0-d2, v1=d1+d2, v2=d2-d1, v3=d1-d3   (d_j = x[2t-1+j], zero-padded)
  M[p][y] = sum_{ky,cin} U[p,ky,cin,:] * V_p[y-1+ky]   (PSUM f32 accum)
  out[2t]   = M0+M1+M2+bias,  out[2t+1] = M1-M2-M3+bias
U = G @ w_row is precomputed on host (f64->bf16), G=[[1,0,0],[.5,.5,.5],
[.5,-.5,.5],[0,0,1]]. PE work drops 9/6 = 1.5x vs direct conv; V production
and the 4-op output transform run on DVE (scalar_tensor_tensor, bias fused).

Host also pre-deinterleaves input columns (even/odd phases, zero-padded to
57+57 per row, bf16) so every DVE read is packed stride-1 bf16 (2 elem/cyc).

Layout per core: 4 strips of 28 output rows; V strip buffers [128,4,30,56]
bf16 per (strip, in-half); 4 PSUM planes of [128,7,56] f32 per group of 7
output rows (8-bank pool = 2 groups in flight). V for strip s+1 is issued
on DVE before the transforms of strip s so the PE never waits at strip
boundaries. Input DMAs ride the scalar-engine ring, outputs the sync ring.
Output is written bf16 and upcast on host.
"""
import numpy as np
import ml_dtypes

from concourse import bacc, bass, mybir, tile
from concourse.bass_utils import run_bass_kernel_spmd

B, C, H, Wd = 16, 256, 112, 112
NCORES = 8
BPC = B // NCORES
RS = 28                  # output rows per strip
NSTRIP = H // RS         # 4
VROWS = RS + 2           # V rows per strip (halo)
NT = 56                  # tiles (output column pairs) per row
NG = 4                   # groups of 7 output rows per strip
GR = RS // NG            # 7
NP = GR * NT             # 392 positions per psum plane
PW = 114                 # packed input row: 57 even + 57 odd phase cols
f32 = mybir.dt.float32
bf16 = mybir.dt.bfloat16
ADD = mybir.AluOpType.add
SUB = mybir.AluOpType.subtract
MULT = mybir.AluOpType.mult


def build(repeat: int = 1, no_in: bool = False, no_out: bool = False,
          no_mm: bool = False, no_v: bool = False, no_tr: bool = False,
          tmode: str = "psum"):
    nc = bacc.Bacc("TRN2", debug=False)
    inp_d = nc.dram_tensor("inp", [BPC, C, H, PW], bf16, kind="ExternalInput").ap()
    w_d = nc.dram_tensor("w", [4, 3, C, C], bf16, kind="ExternalInput").ap()
    bias_d = nc.dram_tensor("bias", [C, 1], f32, kind="ExternalInput").ap()
    out_d = nc.dram_tensor("out", [BPC, C, H, Wd], bf16, kind="ExternalOutput").ap()

    with tile.TileContext(nc) as tc:
        with (
            tc.tile_pool(name="wpool", bufs=1) as wp,
            tc.tile_pool(name="vpool", bufs=1) as vp,
            tc.tile_pool(name="ppool", bufs=4) as ptp,
            tc.tile_pool(name="opool", bufs=4) as op,
            tc.tile_pool(name="tpool", bufs=16) as tp,
            tc.tile_pool(name="pspool", bufs=8, space=bass.MemorySpace.PSUM) as pp,
        ):
            # U weight tiles: lhsT [K=cin-half, M=cout-half] per (p, ky, kh, mh)
            wt = {}
            for p in range(4):
                for ky in range(3):
                    for kh in range(2):
                        for mh in range(2):
                            t = wp.tile([128, 128], bf16, name=f"w{p}{ky}{kh}{mh}")
                            nc.sync.dma_start(
                                t[:], w_d[p, ky, kh * 128:(kh + 1) * 128,
                                          mh * 128:(mh + 1) * 128])
                            wt[p, ky, kh, mh] = t
            biases = []
            for mh in range(2):
                bt = wp.tile([128, 1], f32, name=f"bias{mh}")
                nc.sync.dma_start(bt[:], bias_d[mh * 128:(mh + 1) * 128, :])
                biases.append(bt)

            # V strip buffers (persistent per (s, kh); fully rewritten per use)
            vbufs = {}
            for s in range(NSTRIP):
                for kh in range(2):
                    vbufs[s, kh] = vp.tile([128, 4, VROWS, NT], bf16,
                                           name=f"v{s}{kh}")

            def load_and_v(img, s):
                """DMA packed input rows for strip s and produce its V planes."""
                ys = s * RS
                y0 = max(ys - 1, 0)
                y1 = min(ys + RS + 1, H)
                r0 = y0 - (ys - 1)
                nrows = y1 - y0
                for kh in range(2):
                    pt = ptp.tile([128, VROWS, PW], bf16, name="pt", tag="pt",
                                  bufs=4)
                    if not no_in:
                        nc.scalar.dma_start(
                            pt[:, r0:r0 + nrows, :],
                            inp_d[img, kh * 128:(kh + 1) * 128, y0:y1, :])
                    if s == 0:
                        nc.vector.memset(pt[:, 0:1, :], 0.0)
                    if s == NSTRIP - 1:
                        nc.vector.memset(pt[:, VROWS - 1:VROWS, :], 0.0)
                    if no_v:
                        continue
                    pe0 = pt[:, :, 0:NT]          # pe[t],  t=0..55
                    pe1 = pt[:, :, 1:NT + 1]      # pe[t+1]
                    po0 = pt[:, :, 57:57 + NT]    # po[t]
                    po1 = pt[:, :, 58:58 + NT]    # po[t+1]
                    # plain tensor_tensor (not scalar_tensor_tensor): only the
                    # former is eligible for the DVE 2x packed-bf16 mode
                    v = vbufs[s, kh]
                    nc.vector.tensor_sub(v[:, 0], pe0, pe1)
                    nc.vector.tensor_add(v[:, 1], po0, pe1)
                    nc.vector.tensor_sub(v[:, 2], pe1, po0)
                    nc.vector.tensor_sub(v[:, 3], po0, po1)

            def body():
                seq = [(img, s) for img in range(BPC) for s in range(NSTRIP)]
                load_and_v(*seq[0])
                for idx, (img, s) in enumerate(seq):
                    if idx + 1 < len(seq):
                        load_and_v(*seq[idx + 1])
                    ys = s * RS
                    for mh in range(2):
                        ot = op.tile([128, RS, Wd], bf16, name="ot", tag="ot")
                        oi = ot[:].rearrange("p r (t two) -> p r t two", two=2)
                        for g in range(NG):
                            ps = [pp.tile([128, GR, NT], f32, name="ps",
                                          tag="ps") for _ in range(4)]
                            if not no_mm:
                                for p in range(4):
                                    for c in range(6):
                                        ky, kh = c // 2, c % 2
                                        nc.tensor.matmul(
                                            ps[p][:], wt[p, ky, kh, mh][:],
                                            vbufs[s, kh][:, p,
                                                         GR * g + ky:
                                                         GR * g + ky + GR, :],
                                            start=(c == 0), stop=(c == 5))
                            if no_tr:
                                continue
                            # Act pre-drains M1+bias (walrus forbids >1 PSUM
                            # operand per vector op).
                            a = tp.tile([128, GR, NT], bf16, name="a",
                                        tag="tmp")
                            nc.scalar.activation(
                                a[:], ps[1][:],
                                mybir.ActivationFunctionType.Identity,
                                bias=biases[mh][:])
                            te = tp.tile([128, GR, NT], bf16, name="te",
                                         tag="tmp")
                            to = tp.tile([128, GR, NT], bf16, name="to",
                                         tag="tmp")
                            if tmode == "act":
                                # Act also drains M0/M2/M3 to bf16 SBUF:
                                # frees PSUM early, every DVE op is all-SBUF
                                # packed bf16 (2x mode).
                                mb = []
                                for p in (0, 2, 3):
                                    m = tp.tile([128, GR, NT], bf16,
                                                name=f"m{p}", tag="tmp")
                                    nc.scalar.activation(
                                        m[:], ps[p][:],
                                        mybir.ActivationFunctionType.Copy)
                                    mb.append(m)
                                m0b, m2b, m3b = mb
                                nc.vector.tensor_add(te[:], a[:], m2b[:])
                                nc.vector.tensor_add(
                                    oi[:, GR * g:GR * g + GR, :, 0], te[:],
                                    m0b[:])
                                nc.vector.tensor_sub(to[:], a[:], m2b[:])
                                nc.vector.tensor_sub(
                                    oi[:, GR * g:GR * g + GR, :, 1], to[:],
                                    m3b[:])
                            else:
                                # PSUM-direct: DVE reads each remaining plane
                                # straight from PSUM (1 PSUM operand per op,
                                # 1x rate) - less SBUF traffic, idle Act.
                                nc.vector.tensor_add(te[:], a[:], ps[2][:])
                                nc.vector.tensor_add(
                                    oi[:, GR * g:GR * g + GR, :, 0], te[:],
                                    ps[0][:])
                                nc.vector.tensor_sub(to[:], a[:], ps[2][:])
                                nc.vector.tensor_sub(
                                    oi[:, GR * g:GR * g + GR, :, 1], to[:],
                                    ps[3][:])
                        if not no_out:
                            nc.sync.dma_start(
                                out_d[img, mh * 128:(mh + 1) * 128,
                                      ys:ys + RS, :]
                                .rearrange("p r c -> p (r c)"),
                                ot[:].rearrange("p r c -> p (r c)"))

            if repeat > 1:
                with tc.For_i(0, repeat, 1):
                    body()
            else:
                body()

    nc.compile()
    return nc


_NC = None
_G = np.array([[1, 0, 0], [0.5, 0.5, 0.5], [0.5, -0.5, 0.5], [0, 0, 1]],
              dtype=np.float64)


def _prep(inp, W, bias):
    U = np.einsum("pk,oiyk->pyio", _G, np.asarray(W, dtype=np.float64))
    U = np.ascontiguousarray(U.astype(ml_dtypes.bfloat16))
    x = np.asarray(inp, dtype=np.float32).astype(ml_dtypes.bfloat16)
    p = np.zeros((B, C, H, Wd + 3), dtype=ml_dtypes.bfloat16)
    p[..., 1:113] = x
    pk = np.concatenate([p[..., 0::2][..., :57], p[..., 1::2][..., :57]],
                        axis=-1)
    pk = np.ascontiguousarray(pk)
    bias_r = np.ascontiguousarray(
        np.asarray(bias, dtype=np.float32).reshape(C, 1))
    return pk, U, bias_r


def kernel(inp, W, bias):
    global _NC
    if _NC is None:
        _NC = build()
    pk, U, bias_r = _prep(inp, W, bias)
    in_maps = [
        {"inp": pk[c * BPC:(c + 1) * BPC], "w": U, "bias": bias_r}
        for c in range(NCORES)
    ]
    res = run_bass_kernel_spmd(_NC, in_maps, list(range(NCORES)))
    out = np.concatenate(
        [np.asarray(res.results[c]["out"]) for c in range(NCORES)], axis=0)
    return out.astype(np.float32)
